# revision 1
# baseline (speedup 1.0000x reference)
"""Differentiable Gaussian renderer as a Trainium2 Bass kernel.

Strategy (self-contained; shapes hardcoded from the problem spec):
  - 8 NeuronCores, image row-sharded: core k renders rows [32k, 32k+32).
  - Per core, the 32x256 band is split into 64 pixel tiles of 8x16 = 128
    pixels; each tile's pixels live on the 128 SBUF partitions.
  - Host prep (numpy, float64): project gaussians, depth-sort, and build a
    per-(core,tile) culled gaussian list (precise point-to-rectangle
    mahalanobis culling).  Tiles are packed along the free dimension as
    [sep][g0..gC-1][sep][...] segments, identical layout on all 8 cores
    (per-rank capacity = max over cores), so one NEFF runs SPMD.
  - Device: Q = Gmat.T @ F (one shared [6,128] stationary pixel-polynomial
    matrix, fp32 matmul per PSUM bank), alpha_pre = Exp(Q) on ACT,
    alpha = min(alpha_pre, 0.99), one_minus_alpha, then the front-to-back
    transmittance cumprod is ONE tensor_tensor_scan along the free dim
    (separator columns reset the running product via max with an inject
    vector), w = alpha * T_excl, and per-slot tensor_tensor_reduce against
    replicated per-gaussian colors accumulates the 3 output channels.
  - Host unscrambles the [128, 192] per-core outputs into [3, 256, 256].
"""

import math
import numpy as np

H = W = 256
FX = FY = 300.0
CX = CY = 128.0
NEAR, FAR = 0.01, 100.0
TR, TC = 8, 16          # pixel tile shape (rows x cols); TR*TC == 128
NTY, NTX = 32 // TR, 256 // TC
NSLOTS = NTY * NTX      # 64 tiles per core
NCORES = 8
QCUT = 10.0             # keep (gaussian, tile) if max_tile Q + log(opacity) > -QCUT
F_PAD = -88.0           # Q constant for separator / padding columns -> exp ~ 0

_compile_cache: dict = {}


def _host_prep(positions, scales, rotations, colors, opacities, view_matrix):
    N = positions.shape[0]
    f32 = np.float32

    # ---- depth sort exactly as the fp32 reference does ----
    pts_h32 = np.concatenate(
        [positions.astype(f32), np.ones((N, 1), f32)], axis=1)
    pcam32 = pts_h32 @ view_matrix.astype(f32).T
    x32, y32, z32 = pcam32[:, 0], pcam32[:, 1], pcam32[:, 2]
    depths32 = -z32
    order = np.argsort(depths32, kind="stable")

    # visibility mask in fp32 (must match reference's boundary decisions)
    z_safe32 = (np.clip(np.abs(z32), 0.01, None) *
                np.sign(z32 + f32(1e-8))).astype(f32)
    u32 = (f32(FX) * x32 / -z_safe32 + f32(CX)).astype(f32)
    v32 = (f32(FY) * -y32 / -z_safe32 + f32(CY)).astype(f32)
    vis = ((depths32 > NEAR) & (depths32 < FAR)
           & (u32 > -100) & (u32 < W + 100)
           & (v32 > -100) & (v32 < H + 100))

    # ---- float64 versions of the per-gaussian quantities ----
    pos = positions.astype(np.float64)
    sc = scales.astype(np.float64)
    rot = rotations.astype(np.float64)
    vm = view_matrix.astype(np.float64)
    q = rot / np.linalg.norm(rot, axis=-1, keepdims=True)
    qw, qx, qy, qz = q[:, 0], q[:, 1], q[:, 2], q[:, 3]
    Rm = np.stack([
        1 - 2*qy*qy - 2*qz*qz, 2*qx*qy - 2*qw*qz, 2*qx*qz + 2*qw*qy,
        2*qx*qy + 2*qw*qz, 1 - 2*qx*qx - 2*qz*qz, 2*qy*qz - 2*qw*qx,
        2*qx*qz - 2*qw*qy, 2*qy*qz + 2*qw*qx, 1 - 2*qx*qx - 2*qy*qy,
    ], axis=-1).reshape(N, 3, 3)
    pts = np.concatenate([pos, np.ones((N, 1))], 1) @ vm.T
    X, Y, Z = pts[:, 0], pts[:, 1], pts[:, 2]
    Rcam = np.einsum('ij,njk->nik', vm[:3, :3], Rm)
    RS = Rcam * sc[:, None, :]
    cov3d = RS @ np.swapaxes(RS, -1, -2)
    z_safe = np.clip(np.abs(Z), 0.01, None) * np.sign(Z + 1e-8)
    z2 = z_safe * z_safe
    J = np.zeros((N, 2, 3))
    J[:, 0, 0] = FX / -z_safe
    J[:, 0, 2] = FX * X / z2
    J[:, 1, 1] = FY / z_safe
    J[:, 1, 2] = FY * Y / z2
    cov2d = np.einsum('nij,njk,nlk->nil', J, cov3d, J)
    u = FX * X / -z_safe + CX
    v = FY * -Y / -z_safe + CY

    # sort everything front-to-back
    u, v, vis = u[order], v[order], vis[order]
    cov2d = cov2d[order]
    opa = opacities.astype(np.float64)[order]
    cols = colors.astype(np.float64)[order]

    a = cov2d[:, 0, 0] + 1e-4
    b = cov2d[:, 0, 1]
    c = cov2d[:, 1, 1] + 1e-4
    det = a * c - b * b
    ia2 = -0.5 * c / det
    ib2 = b / det
    ic2 = -0.5 * a / det
    keepable = vis & (opa > 0)
    logo = np.where(keepable, np.log(np.maximum(opa, 1e-300)), -1e9)


    # ---- precise per-(core,tile) culling ----
    # max over the tile rectangle of the concave quadratic Q(p); exact via
    # edge maximization + interior check.
    def qmax_tile(y0, x0):
        inside = (u >= x0) & (u <= x0 + TC - 1) & (v >= y0) & (v <= y0 + TR - 1)
        best = np.full(N, -np.inf)
        for xe in (x0, x0 + TC - 1):
            dx = xe - u
            dy_cl = np.clip(-ib2 * dx / (2 * ic2), y0 - v, y0 + TR - 1 - v)
            best = np.maximum(best, ia2*dx*dx + ib2*dx*dy_cl + ic2*dy_cl*dy_cl)
        for ye in (y0, y0 + TR - 1):
            dy = ye - v
            dx_cl = np.clip(-ib2 * dy / (2 * ia2), x0 - u, x0 + TC - 1 - u)
            best = np.maximum(best, ia2*dx_cl*dx_cl + ib2*dx_cl*dy + ic2*dy*dy)
        return np.where(inside, 0.0, best)

    keep = np.zeros((NCORES, NSLOTS, N), bool)
    for core in range(NCORES):
        for ti in range(NSLOTS):
            y0 = core * 32 + (ti // NTX) * TR
            x0 = (ti % NTX) * TC
            keep[core, ti] = keepable & (qmax_tile(y0, x0) + logo > -QCUT)

    counts = keep.sum(axis=2)                      # [8, 64]
    slot_order = np.argsort(-counts, axis=1, kind="stable")  # tiles by count desc
    counts_sorted = np.take_along_axis(counts, slot_order, axis=1)
    caps = counts_sorted.max(axis=0).astype(np.int64)        # [64] rank max
    # pack slots as [sep][g...] segments, never crossing a 512-col PSUM bank
    # boundary (keeps every consumer instruction's semaphore-wait count tiny)
    offs = np.zeros(NSLOTS, np.int64)
    col0 = 0
    for r in range(NSLOTS):
        seg = int(caps[r]) + 1
        if (col0 % 512) + seg > 512:
            col0 = (col0 // 512 + 1) * 512
        offs[r] = col0
        col0 += seg
    L = int(col0)
    # color-matmul blocks: for each 128-col block of L, the (rank-consecutive)
    # slots whose gaussian columns intersect it, plus a block-sparse color
    # matrix [128, 3k] mapping block rows to slot color columns
    nblocks = -(-L // 128)
    blocks = []          # (b, m, j0, j1, cb_off)
    cb_parts = [[] for _ in range(NCORES)]
    cb_off = 0
    for bb in range(nblocks):
        lo, hi = bb * 128, min(bb * 128 + 128, L)
        m = hi - lo
        js = [j for j in range(NSLOTS) if caps[j] > 0
              and offs[j] + 1 < hi and offs[j] + 1 + caps[j] > lo]
        if not js:
            continue
        j0, j1 = min(js), max(js)
        assert js == list(range(j0, j1 + 1))
        k = j1 - j0 + 1
        blocks.append((bb, m, j0, j1, cb_off))
        cb_off += 3 * k
    CB = max(cb_off, 1)
    # ---- packed per-core arrays ----
    fmat = np.zeros((NCORES, 6, L), f32)
    fmat[:, 5, :] = F_PAD
    colblk = np.zeros((NCORES, 128, CB), f32)

    for core in range(NCORES):
        for r in range(NSLOTS):
            ti = int(slot_order[core, r])
            n = int(counts[core, ti])
            if n == 0:
                continue
            y0 = core * 32 + (ti // NTX) * TR
            x0 = (ti % NTX) * TC
            x0c = x0 + (TC - 1) / 2.0
            y0c = y0 + (TR - 1) / 2.0
            g = np.where(keep[core, ti])[0]        # sorted (front-to-back)
            up = u[g] - x0c
            vp = v[g] - y0c
            s = int(offs[r]) + 1
            fmat[core, 0, s:s+n] = ia2[g]
            fmat[core, 1, s:s+n] = ib2[g]
            fmat[core, 2, s:s+n] = ic2[g]
            fmat[core, 3, s:s+n] = -2*ia2[g]*up - ib2[g]*vp
            fmat[core, 4, s:s+n] = -2*ic2[g]*vp - ib2[g]*up
            fmat[core, 5, s:s+n] = (ia2[g]*up*up + ib2[g]*up*vp
                                    + ic2[g]*vp*vp + logo[g])
            # scatter colors into the block-sparse color matrices
            for bb, m, j0, j1, cbo in blocks:
                lo, hi = bb * 128, bb * 128 + m
                a0 = max(s, lo)
                a1 = min(s + n, hi)
                if a0 >= a1 or not (j0 <= r <= j1):
                    continue
                rows = np.arange(a0 - lo, a1 - lo)
                colblk[core, rows, cbo + 3 * (r - j0) + 0] = cols[g[a0-s:a1-s], 0]
                colblk[core, rows, cbo + 3 * (r - j0) + 1] = cols[g[a0-s:a1-s], 1]
                colblk[core, rows, cbo + 3 * (r - j0) + 2] = cols[g[a0-s:a1-s], 2]

    # pixel polynomial matrix, shared by every tile and core
    dr, dc = np.divmod(np.arange(128), TC)
    gx = (dc - (TC - 1) / 2.0).astype(f32)
    gy = (dr - (TR - 1) / 2.0).astype(f32)
    gm = np.stack([gx*gx, gx*gy, gy*gy, gx, gy, np.ones(128, f32)]).astype(f32)

    # fp16 split of F: F = hi + lo recovers ~21 mantissa bits; the pixel
    # polynomial matrix gm is exact in fp16 (ints, quantum 0.25). Guarded by
    # magnitude: fp16 max is 65504, and term-cancellation error scales with
    # |F|, so fall back to fp32 matmuls when coefficients are large.
    use_f16 = bool(np.abs(fmat).max() < 16000.0)
    inj = np.zeros(L, np.float32)
    inj[offs] = 1.0
    inj_rep = np.broadcast_to(inj, (128, L)).copy()

    in_maps = []
    ident = np.eye(128, dtype=np.float16)
    for core in range(NCORES):
        if use_f16:
            fhi = fmat[core].astype(np.float16)
            flo = (fmat[core].astype(np.float64)
                   - fhi.astype(np.float64)).astype(np.float16)
            fmat_all = np.concatenate(
                [gm.astype(np.float16), fhi, flo], axis=1)
        else:
            fmat_all = np.concatenate([gm, fmat[core]], axis=1)
        in_maps.append({
            "fmat": np.ascontiguousarray(fmat_all),
            "colblk": np.ascontiguousarray(colblk[core].astype(np.float16)),
            "ident": ident,
            "inj": inj_rep,
        })
    return (in_maps, L, tuple(int(x) for x in caps), offs, slot_order,
            blocks, CB, use_f16)


def _build_program(L, caps, offs, blocks, CB, use_f16):
    import concourse.bacc as bacc
    import concourse.mybir as mybir
    import math
    from concourse.tile import TileContext
    from concourse.mybir import AluOpType

    f32 = mybir.dt.float32
    f16 = mybir.dt.float16
    fdt = f16 if use_f16 else f32
    fm_cols = (128 + 2 * L) if use_f16 else (128 + L)
    nc = bacc.Bacc("TRN2", target_bir_lowering=False)
    f_d = nc.dram_tensor("fmat", [6, fm_cols], fdt, kind="ExternalInput")
    cb_d = nc.dram_tensor("colblk", [128, CB], f16, kind="ExternalInput")
    id_d = nc.dram_tensor("ident", [128, 128], f16, kind="ExternalInput")
    inj_d = nc.dram_tensor("inj", [128, L], f32, kind="ExternalInput")
    out_d = nc.dram_tensor("out", [128, 3 * NSLOTS], f32, kind="ExternalOutput")

    banks = []
    c0 = 0
    while c0 < L:
        banks.append((c0, min(c0 + 512, L)))
        c0 += 512
    blocks_by_bank: dict[int, list] = {}
    for blk in blocks:
        blocks_by_bank.setdefault(blk[0] // 4, []).append(blk)

    LN99 = float(math.log(0.99))

    with TileContext(nc) as tc:
        with (
            tc.tile_pool(name="const", bufs=1) as cpool,
            tc.tile_pool(name="wts", bufs=3) as wpool,
            tc.tile_pool(name="psum", bufs=3, space="PSUM") as ppool,
            tc.tile_pool(name="trps", bufs=3, space="PSUM") as tpool,
            tc.tile_pool(name="colps", bufs=1, space="PSUM") as opool,
        ):
            fm_all = cpool.tile([6, fm_cols], fdt)
            nc.sync.dma_start(fm_all[:, :], f_d[:, :])
            gm = fm_all[:, 0:128]
            fhi = fm_all[:, 128:128 + L]
            flo = fm_all[:, 128 + L:128 + 2 * L] if use_f16 else None
            cb = cpool.tile([128, CB], f16)
            nc.sync.dma_start(cb[:, :], cb_d[:, :])
            ident = cpool.tile([128, 128], f16)
            nc.sync.dma_start(ident[:, :], id_d[:, :])
            inj = cpool.tile([128, L], f32)
            nc.sync.dma_start(inj[:, :], inj_d[:, :])

            alphat = cpool.tile([128, L], f32)
            omap = cpool.tile([128, L], f32)
            Tt = cpool.tile([128, L], f32)
            wt = cpool.tile([128, L], f16)
            colb = cpool.tile([128, 3 * NSLOTS], f32)
            colps = opool.tile([128, 3 * NSLOTS], f32)

            nc.vector.memset(colps[:, :], 0.0)
            nc.vector.memset(wt[:, 0:1], 0.0)

            for bi, (c0, c1) in enumerate(banks):
                n = c1 - c0
                ps = ppool.tile([128, 512], f32, tag="ps", name="ps")
                if use_f16:
                    nc.tensor.matmul(ps[:, :n], gm[:, :], fhi[:, c0:c1],
                                     start=True, stop=False)
                    nc.tensor.matmul(ps[:, :n], gm[:, :], flo[:, c0:c1],
                                     start=False, stop=True)
                else:
                    nc.tensor.matmul(ps[:, :n], gm[:, :], fhi[:, c0:c1],
                                     start=True, stop=True)
                # clamp in Q-space: alpha = exp(min(Q, ln .99)) == min(exp(Q), .99)
                # and then 1 - alpha >= 0.01 automatically (no extra clamp pass)
                nc.vector.tensor_scalar(ps[:, :n], ps[:, :n], LN99, None,
                                        AluOpType.min)
                nc.scalar.activation(alphat[:, c0:c1], ps[:, :n],
                                     mybir.ActivationFunctionType.Exp)
                nc.vector.tensor_scalar(omap[:, c0:c1], alphat[:, c0:c1],
                                        -1.0, 1.0, AluOpType.mult,
                                        AluOpType.add)
                init = 0.0 if bi == 0 else Tt[:, c0 - 1: c0]
                nc.vector.tensor_tensor_scan(Tt[:, c0:c1], omap[:, c0:c1],
                                             inj[:, c0:c1], init,
                                             AluOpType.mult, AluOpType.max)
                # exclusive transmittance: w[:, c] = alpha[:, c] * T[:, c-1]
                # (wt[:, 0] is memset once; all other bank-start columns read
                # T across the bank boundary, which the scan chain provides)
                w0 = c0 + 1 if bi == 0 else c0
                nc.vector.tensor_tensor(wt[:, w0: c1],
                                        alphat[:, w0: c1],
                                        Tt[:, w0 - 1: c1 - 1], AluOpType.mult)
                # color: per 128-col block, transpose w on the TensorEngine,
                # then one small matmul against the block-sparse color matrix
                # accumulates every slot's [128px, 3] color into one PSUM bank
                # pair blocks two-per-PSUM-bank: one ACT drain serves two
                # transposes, then one color matmul per block
                blks = blocks_by_bank.get(bi, [])
                for p0 in range(0, len(blks), 2):
                    pair = blks[p0:p0 + 2]
                    trp = tpool.tile([128, 256], f16, tag="trp", name="trp")
                    wT = wpool.tile([128, 256], f16, tag="wT", name="wT")
                    span = 0
                    for t, (bb, m, j0, j1, cbo) in enumerate(pair):
                        lo = bb * 128
                        nc.tensor.transpose(trp[:m, 128 * t:128 * t + 128],
                                            wt[:, lo:lo + m], ident[:, :])
                        span = 128 * t + 128
                    nc.scalar.copy(wT[:, :span], trp[:, :span])
                    for t, (bb, m, j0, j1, cbo) in enumerate(pair):
                        k3 = 3 * (j1 - j0 + 1)
                        nc.tensor.matmul(colps[:, 3 * j0: 3 * j0 + k3],
                                         wT[:m, 128 * t:128 * t + 128],
                                         cb[:m, cbo: cbo + k3],
                                         start=False, stop=False,
                                         skip_group_check=True)

            nc.vector.tensor_scalar(colb[:, :], colps[:, :], 0.0, 1.0,
                                    AluOpType.max, AluOpType.min)
            nc.sync.dma_start(out_d[:, :], colb[:, :])
    nc.finalize()
    return nc


def _assemble(results, slot_order):
    out = np.zeros((3, H, W), np.float32)
    dr, dc = np.divmod(np.arange(128), TC)
    for core in range(NCORES):
        o = results[core]["out"]          # [128, 192]
        for r in range(NSLOTS):
            ti = int(slot_order[core, r])
            y0 = core * 32 + (ti // NTX) * TR
            x0 = (ti % NTX) * TC
            for ch in range(3):
                out[ch, y0 + dr, x0 + dc] = o[:, 3 * r + ch]
    return out


def _run(inputs, trace=False, trace_cores=None):
    (in_maps, L, caps, offs, slot_order, blocks, CB, use_f16) = _host_prep(
        inputs["positions"], inputs["scales"], inputs["rotations"],
        inputs["colors"], inputs["opacities"], inputs["view_matrix"])

    key = (L, caps, tuple(int(o) for o in offs), use_f16)
    if key not in _compile_cache:
        _compile_cache[key] = _build_program(L, caps, offs, blocks, CB, use_f16)
    nc = _compile_cache[key]

    from concourse.bass_utils import run_bass_kernel_spmd
    kw = {}
    if trace:
        kw = dict(trace=True,
                  trace_cores=trace_cores or list(range(NCORES)))
    res = run_bass_kernel_spmd(nc, in_maps, core_ids=list(range(NCORES)), **kw)
    return _assemble(res.results, slot_order), res


def kernel(**inputs) -> np.ndarray:
    out, _ = _run(inputs, trace=False)
    return out



# revision 7
# speedup vs baseline: 1.6699x; 1.6699x over previous
"""Differentiable Gaussian renderer as a Trainium2 Bass kernel.

Strategy (self-contained; shapes hardcoded from the problem spec):
  - 8 NeuronCores. The 256x256 image is cut into 512 tiles of 8x16 = 128
    pixels (the SBUF partition count). Tiles are assigned to cores by a
    count-balancing serpentine over per-tile culled-gaussian counts, so
    the SPMD rank-max slot capacities are near the mean (not the max of
    a spatial band).
  - Host prep (numpy, float64): project gaussians, depth-sort, precise
    point-to-rectangle mahalanobis culling per (tile), pack each core's
    64 slots as [sep][g0..gC-1] segments along the free dimension
    (identical capacities on all cores -> one NEFF runs SPMD), never
    crossing a 512-col PSUM bank.
  - Device, per 512-col bank:
      Q      = gm.T @ F          (one fp32r matmul, PSUM)
      alpha  = Exp(Q)            (ACT, in-place in PSUM)
      omr    = 1 - alpha         (ACT Identity scale=-1 bias=+1 -> f16 SBUF)
      om     = max(omr, 0.01)    (DVE in-place f16, 4x mode)
      T      = scan(om, inj)     (DVE tensor_tensor_scan mult/max; inj
                                  resets T to 1 at separators; chained
                                  across banks via init)
    Colors use Abel summation: sum_c w[c]*col[c] == sum_c T[c]*dcol[c]
    with dcol[c] = col[c+1]-col[c], so T is transposed directly
    (per 128-col block on the PE) and one small matmul per block
    accumulates into the output PSUM; no explicit w tensor.
  - Output is clipped and DMA'd in two chunks so the first chunk
    overlaps the tail of compute. Host unscrambles [128, 192] per-core
    outputs into [3, 256, 256].
"""

import math
import numpy as np

H = W = 256
FX = FY = 300.0
CX = CY = 128.0
NEAR, FAR = 0.01, 100.0
TR, TC = 8, 16          # pixel tile shape (rows x cols); TR*TC == 128
NTY, NTX = H // TR, W // TC     # 32 x 16 = 512 tiles
NSLOTS = 64             # slots per core (512 tiles / 8 cores)
NCORES = 8
QCUT = 6.0              # keep (gaussian, tile) if max_tile Q + log(opacity) > -QCUT
F_PAD = -88.0           # Q constant for separator / padding columns -> exp ~ 0
BANK = 512

_compile_cache: dict = {}


def _host_prep(positions, scales, rotations, colors, opacities, view_matrix):
    N = positions.shape[0]
    f32 = np.float32
    f16 = np.float16

    # ---- depth sort exactly as the fp32 reference does ----
    pts_h32 = np.concatenate(
        [positions.astype(f32), np.ones((N, 1), f32)], axis=1)
    pcam32 = pts_h32 @ view_matrix.astype(f32).T
    x32, y32, z32 = pcam32[:, 0], pcam32[:, 1], pcam32[:, 2]
    depths32 = -z32
    order = np.argsort(depths32, kind="stable")

    # visibility mask in fp32 (must match reference's boundary decisions)
    z_safe32 = (np.clip(np.abs(z32), 0.01, None) *
                np.sign(z32 + f32(1e-8))).astype(f32)
    u32 = (f32(FX) * x32 / -z_safe32 + f32(CX)).astype(f32)
    v32 = (f32(FY) * -y32 / -z_safe32 + f32(CY)).astype(f32)
    vis = ((depths32 > NEAR) & (depths32 < FAR)
           & (u32 > -100) & (u32 < W + 100)
           & (v32 > -100) & (v32 < H + 100))

    # ---- float64 versions of the per-gaussian quantities ----
    pos = positions.astype(np.float64)
    sc = scales.astype(np.float64)
    rot = rotations.astype(np.float64)
    vm = view_matrix.astype(np.float64)
    q = rot / np.linalg.norm(rot, axis=-1, keepdims=True)
    qw, qx, qy, qz = q[:, 0], q[:, 1], q[:, 2], q[:, 3]
    Rm = np.stack([
        1 - 2*qy*qy - 2*qz*qz, 2*qx*qy - 2*qw*qz, 2*qx*qz + 2*qw*qy,
        2*qx*qy + 2*qw*qz, 1 - 2*qx*qx - 2*qz*qz, 2*qy*qz - 2*qw*qx,
        2*qx*qz - 2*qw*qy, 2*qy*qz + 2*qw*qx, 1 - 2*qx*qx - 2*qy*qy,
    ], axis=-1).reshape(N, 3, 3)
    pts = np.concatenate([pos, np.ones((N, 1))], 1) @ vm.T
    X, Y, Z = pts[:, 0], pts[:, 1], pts[:, 2]
    Rcam = np.einsum('ij,njk->nik', vm[:3, :3], Rm)
    RS = Rcam * sc[:, None, :]
    cov3d = RS @ np.swapaxes(RS, -1, -2)
    z_safe = np.clip(np.abs(Z), 0.01, None) * np.sign(Z + 1e-8)
    z2 = z_safe * z_safe
    J = np.zeros((N, 2, 3))
    J[:, 0, 0] = FX / -z_safe
    J[:, 0, 2] = FX * X / z2
    J[:, 1, 1] = FY / z_safe
    J[:, 1, 2] = FY * Y / z2
    cov2d = np.einsum('nij,njk,nlk->nil', J, cov3d, J)
    u = FX * X / -z_safe + CX
    v = FY * -Y / -z_safe + CY

    # sort everything front-to-back
    u, v, vis = u[order], v[order], vis[order]
    cov2d = cov2d[order]
    opa = opacities.astype(np.float64)[order]
    cols = colors.astype(np.float64)[order]

    a = cov2d[:, 0, 0] + 1e-4
    b = cov2d[:, 0, 1]
    c = cov2d[:, 1, 1] + 1e-4
    det = a * c - b * b
    ia2 = -0.5 * c / det
    ib2 = b / det
    ic2 = -0.5 * a / det
    keepable = vis & (opa > 0)
    logo = np.where(keepable, np.log(np.maximum(opa, 1e-300)), -1e9)

    # ---- precise per-tile culling ----
    # max over the tile rectangle of the concave quadratic Q(p); exact via
    # edge maximization + interior check.
    def qmax_tile(y0, x0):
        inside = (u >= x0) & (u <= x0 + TC - 1) & (v >= y0) & (v <= y0 + TR - 1)
        best = np.full(N, -np.inf)
        for xe in (x0, x0 + TC - 1):
            dx = xe - u
            dy_cl = np.clip(-ib2 * dx / (2 * ic2), y0 - v, y0 + TR - 1 - v)
            best = np.maximum(best, ia2*dx*dx + ib2*dx*dy_cl + ic2*dy_cl*dy_cl)
        for ye in (y0, y0 + TR - 1):
            dy = ye - v
            dx_cl = np.clip(-ib2 * dy / (2 * ia2), x0 - u, x0 + TC - 1 - u)
            best = np.maximum(best, ia2*dx_cl*dx_cl + ib2*dx_cl*dy + ic2*dy*dy)
        return np.where(inside, 0.0, best)

    NT = NTY * NTX                       # 512 global tiles
    keep = np.zeros((NT, N), bool)
    for t in range(NT):
        ty, tx = divmod(t, NTX)
        keep[t] = keepable & (qmax_tile(ty * TR, tx * TC) + logo > -QCUT)
    counts = keep.sum(axis=1)            # [512]

    # ---- balanced tile -> core assignment (serpentine over sorted counts) ----
    tile_order = np.argsort(-counts, kind="stable")
    core_tiles = np.zeros((NCORES, NSLOTS), np.int64)   # rank r -> global tile
    for r in range(NSLOTS):
        blk = tile_order[r * NCORES:(r + 1) * NCORES]
        cores = range(NCORES) if r % 2 == 0 else range(NCORES - 1, -1, -1)
        for t, core in zip(blk, cores):
            core_tiles[core, r] = t
    # rank-wise capacities (max over cores of r-th largest count)
    caps = np.zeros(NSLOTS, np.int64)
    for r in range(NSLOTS):
        caps[r] = max(int(counts[core_tiles[core, r]]) for core in range(NCORES))

    # pack slots as [sep][g...] segments, never crossing a 512-col PSUM bank
    offs = np.full(NSLOTS, -1, np.int64)
    col0 = 0
    for r in range(NSLOTS):
        if caps[r] == 0:
            continue
        seg = int(caps[r]) + 1
        if (col0 % BANK) + seg > BANK:
            col0 = (col0 // BANK + 1) * BANK
        offs[r] = col0
        col0 += seg
    L = int(col0)
    L += L % 2          # fp32r matmul needs an even moving free dim

    # color-matmul blocks: for each 128-col block of L, the (rank-consecutive)
    # slots whose [sep..sep+cap] interval intersects it, plus a block-sparse
    # color-difference matrix [128, 3k] (Abel summation: dcol = col[c+1]-col[c])
    nblocks = -(-L // 128)
    blocks = []          # (bb, m, j0, j1, cb_off)
    cb_off = 0
    for bb in range(nblocks):
        lo, hi = bb * 128, min(bb * 128 + 128, L)
        m = hi - lo
        js = [j for j in range(NSLOTS) if caps[j] > 0
              and offs[j] < hi and offs[j] + caps[j] >= lo]
        if not js:
            continue
        j0, j1 = min(js), max(js)
        assert js == list(range(j0, j1 + 1))
        blocks.append((bb, m, j0, j1, cb_off))
        cb_off += 3 * (j1 - j0 + 1)
    CB = max(cb_off, 1)

    # ---- packed per-core arrays ----
    fmat = np.zeros((NCORES, 6, L), f32)
    fmat[:, 5, :] = F_PAD
    colfull = np.zeros((NCORES, L + 1, 3), f32)   # per-position colors (+1 pad)

    for core in range(NCORES):
        for r in range(NSLOTS):
            if caps[r] == 0:
                continue
            t = int(core_tiles[core, r])
            n = int(counts[t])
            if n == 0:
                continue
            ty, tx = divmod(t, NTX)
            y0, x0 = ty * TR, tx * TC
            x0c = x0 + (TC - 1) / 2.0
            y0c = y0 + (TR - 1) / 2.0
            g = np.where(keep[t])[0]        # sorted (front-to-back)
            up = u[g] - x0c
            vp = v[g] - y0c
            s = int(offs[r]) + 1
            fmat[core, 0, s:s+n] = ia2[g]
            fmat[core, 1, s:s+n] = ib2[g]
            fmat[core, 2, s:s+n] = ic2[g]
            fmat[core, 3, s:s+n] = -2*ia2[g]*up - ib2[g]*vp
            fmat[core, 4, s:s+n] = -2*ic2[g]*vp - ib2[g]*up
            fmat[core, 5, s:s+n] = (ia2[g]*up*up + ib2[g]*up*vp
                                    + ic2[g]*vp*vp + logo[g])
            colfull[core, s:s+n, :] = cols[g, :]

    # Abel: dcol[c] = col[c+1] - col[c]
    dcol = colfull[:, 1:, :] - colfull[:, :-1, :]       # [NCORES, L, 3]
    colblk = np.zeros((NCORES, 128, CB), f16)
    for bb, m, j0, j1, cbo in blocks:
        lo = bb * 128
        for j in range(j0, j1 + 1):
            a0 = max(int(offs[j]), lo)
            a1 = min(int(offs[j]) + int(caps[j]) + 1, lo + m)
            if a0 >= a1:
                continue
            rows = np.arange(a0 - lo, a1 - lo)
            colblk[:, rows, cbo + 3*(j - j0):cbo + 3*(j - j0) + 3] = \
                dcol[:, a0:a1, :].astype(f16)

    # pixel polynomial matrix, shared by every tile and core
    dr, dc = np.divmod(np.arange(128), TC)
    gx = (dc - (TC - 1) / 2.0).astype(f32)
    gy = (dr - (TR - 1) / 2.0).astype(f32)
    gm = np.stack([gx*gx, gx*gy, gy*gy, gx, gy, np.ones(128, f32)]).astype(f32)

    inj = np.zeros(L, f16)
    inj[offs[offs >= 0]] = 1.0
    inj_rep = np.broadcast_to(inj, (128, L)).copy()

    ident = np.eye(128, dtype=f16)
    in_maps = []
    for core in range(NCORES):
        fmat_all = np.concatenate([gm, fmat[core]], axis=1)   # [6, 128+L] f32
        in_maps.append({
            "fmat": np.ascontiguousarray(fmat_all),
            "colblk": np.ascontiguousarray(colblk[core]),
            "ident": ident,
            "inj": inj_rep,
        })
    caps_t = tuple(int(x) for x in caps)
    offs_t = tuple(int(x) for x in offs)
    return in_maps, L, caps_t, offs_t, core_tiles, blocks, CB


def _choose_split(L, blocks):
    """Pick a clean (non-straddling) block boundary near 65% of L for the
    two-chunk output; returns (split_block_index, j_split) or None."""
    best = None
    for k in range(len(blocks) - 1):
        bb, m, j0, j1, cbo = blocks[k]
        nj0 = blocks[k + 1][2]
        if nj0 > j1:                      # clean boundary
            pos = (bb * 128 + m) / L
            score = abs(pos - 0.65)
            if best is None or score < best[0]:
                best = (score, k, j1 + 1)
    if best is None:
        return None
    return best[1], best[2]


def _build_program(L, caps, offs, blocks, CB):
    import concourse.bacc as bacc
    import concourse.mybir as mybir
    from concourse.tile import TileContext
    from concourse.mybir import AluOpType

    f32 = mybir.dt.float32
    f32r = mybir.dt.float32r
    f16 = mybir.dt.float16
    nc = bacc.Bacc("TRN2", target_bir_lowering=False)
    f_d = nc.dram_tensor("fmat", [6, 128 + L], f32r, kind="ExternalInput")
    cb_d = nc.dram_tensor("colblk", [128, CB], f16, kind="ExternalInput")
    id_d = nc.dram_tensor("ident", [128, 128], f16, kind="ExternalInput")
    inj_d = nc.dram_tensor("inj", [128, L], f16, kind="ExternalInput")
    out_d = nc.dram_tensor("out", [128, 3 * NSLOTS], f32, kind="ExternalOutput")

    banks = []
    c0 = 0
    while c0 < L:
        banks.append((c0, min(c0 + BANK, L)))
        c0 += BANK
    blocks_by_bank: dict[int, list] = {}
    for blk in blocks:
        blocks_by_bank.setdefault(blk[0] // 4, []).append(blk)

    split = _choose_split(L, blocks)
    if split is not None:
        split_k, j_split = split
    else:
        split_k, j_split = len(blocks) - 1, NSLOTS
    # chunk A: slots [0, j_split) finalized after block index split_k
    # chunk B: slots [j_split, NSLOTS)

    with TileContext(nc) as tc:
        with (
            tc.tile_pool(name="const", bufs=1) as cpool,
            tc.tile_pool(name="wts", bufs=3) as wpool,
            tc.tile_pool(name="psum", bufs=3, space="PSUM") as ppool,
            tc.tile_pool(name="trps", bufs=2, space="PSUM") as tpool,
            tc.tile_pool(name="colps", bufs=1, space="PSUM") as opool,
        ):
            fm_all = cpool.tile([6, 128 + L], f32r)
            nc.sync.dma_start(fm_all[:, :], f_d[:, :])
            gm = fm_all[:, 0:128]
            fmat = fm_all[:, 128:128 + L]
            ident = cpool.tile([128, 128], f16)
            nc.scalar.dma_start(ident[:, :], id_d[:, :])
            cb = cpool.tile([128, CB], f16)
            nc.scalar.dma_start(cb[:, :], cb_d[:, :])
            inj = cpool.tile([128, L], f16)
            for c0, c1 in banks:
                nc.sync.dma_start(inj[:, c0:c1], inj_d[:, c0:c1])

            om = cpool.tile([128, L], f16)
            Tt = cpool.tile([128, L], f16)
            colb = cpool.tile([128, 3 * NSLOTS], f32)
            nA = 3 * j_split
            nB = 3 * NSLOTS - nA
            colpsA = opool.tile([128, max(nA, 1)], f32, name="colpsA")
            colpsB = opool.tile([128, max(nB, 1)], f32, name="colpsB")
            nc.vector.memset(colpsA[:, :], 0.0)
            nc.vector.memset(colpsB[:, :], 0.0)

            def col_mm(bidx, blk, wT, toff):
                bb, m, j0, j1, cbo = blk
                k3 = 3 * (j1 - j0 + 1)
                # split the rhs columns between the two output chunks
                lo_j, hi_j = j0, j1 + 1
                if lo_j < j_split:
                    hj = min(hi_j, j_split)
                    k3a = 3 * (hj - lo_j)
                    nc.tensor.matmul(colpsA[:, 3 * lo_j:3 * lo_j + k3a],
                                     wT[:m, toff:toff + 128],
                                     cb[:m, cbo:cbo + k3a],
                                     start=False, stop=False,
                                     skip_group_check=True)
                if hi_j > j_split:
                    lj = max(lo_j, j_split)
                    cbo2 = cbo + 3 * (lj - j0)
                    k3b = 3 * (hi_j - lj)
                    nc.tensor.matmul(
                        colpsB[:, 3 * (lj - j_split):3 * (lj - j_split) + k3b],
                        wT[:m, toff:toff + 128],
                        cb[:m, cbo2:cbo2 + k3b],
                        start=False, stop=False,
                        skip_group_check=True)

            LN99 = float(math.log(0.99))
            drain_on_vector = True
            blk_idx = 0
            for bi, (c0, c1) in enumerate(banks):
                n = c1 - c0
                ps = ppool.tile([128, BANK], f32, tag="ps", name="ps")
                nc.tensor.matmul(ps[:, :n],
                                 gm[:, :],
                                 fmat[:, c0:c1],
                                 start=True, stop=True)
                # alpha = exp(Q), in place in PSUM
                nc.scalar.activation(ps[:, :n], ps[:, :n],
                                     mybir.ActivationFunctionType.Exp)
                # omr = 1 - alpha  (ACT affine: Identity(-1*x + 1)) -> f16 SBUF
                nc.scalar.activation(om[:, c0:c1], ps[:, :n],
                                     mybir.ActivationFunctionType.Identity,
                                     bias=1.0, scale=-1.0)
                # om = max(omr, 0.01), in place (f16 4x mode)
                nc.vector.tensor_scalar(om[:, c0:c1], om[:, c0:c1], 0.01,
                                        None, AluOpType.max)
                init = 0.0 if bi == 0 else Tt[:, c0 - 1: c0]
                nc.vector.tensor_tensor_scan(Tt[:, c0:c1], om[:, c0:c1],
                                             inj[:, c0:c1], init,
                                             AluOpType.mult, AluOpType.max)
                # color path: transpose T per 128-col block (pairs share one
                # PSUM tile + one drain), then small matmuls accumulate
                blks = blocks_by_bank.get(bi, [])
                for p0 in range(0, len(blks), 2):
                    pair = blks[p0:p0 + 2]
                    trp = tpool.tile([128, 256], f16, tag="trp", name="trp")
                    wT = wpool.tile([128, 256], f16, tag="wT", name="wT")
                    span = 0
                    for t, (bb, m, j0, j1, cbo) in enumerate(pair):
                        lo = bb * 128
                        nc.tensor.transpose(trp[:m, 128 * t:128 * t + 128],
                                            Tt[:, lo:lo + m], ident[:, :])
                        span = 128 * t + 128
                    if drain_on_vector:
                        nc.vector.tensor_copy(wT[:, :span], trp[:, :span])
                    else:
                        nc.scalar.copy(wT[:, :span], trp[:, :span])
                    drain_on_vector = not drain_on_vector
                    for t, blk in enumerate(pair):
                        col_mm(blk_idx, blk, wT, 128 * t)
                        blk_idx += 1
                        if blk_idx == split_k + 1 and nA > 0:
                            # chunk A final: clip + DMA now
                            nc.vector.tensor_scalar(
                                colb[:, :nA], colpsA[:, :nA], 0.0, 1.0,
                                AluOpType.max, AluOpType.min)
                            nc.sync.dma_start(out_d[:, :nA], colb[:, :nA])

            if nB > 0:
                nc.vector.tensor_scalar(colb[:, nA:], colpsB[:, :nB], 0.0, 1.0,
                                        AluOpType.max, AluOpType.min)
                nc.sync.dma_start(out_d[:, nA:], colb[:, nA:])
            if split_k + 1 > len(blocks) and nA > 0:
                raise AssertionError("chunk A never finalized")
    nc.finalize()
    return nc


def _assemble(results, core_tiles, caps):
    out = np.zeros((3, H, W), np.float32)
    dr, dc = np.divmod(np.arange(128), TC)
    for core in range(NCORES):
        o = results[core]["out"]          # [128, 192]
        for r in range(NSLOTS):
            if caps[r] == 0:
                continue
            t = int(core_tiles[core, r])
            ty, tx = divmod(t, NTX)
            y0, x0 = ty * TR, tx * TC
            for ch in range(3):
                out[ch, y0 + dr, x0 + dc] = o[:, 3 * r + ch]
    return out


def _run(inputs, trace=False, trace_cores=None):
    in_maps, L, caps, offs, core_tiles, blocks, CB = _host_prep(
        inputs["positions"], inputs["scales"], inputs["rotations"],
        inputs["colors"], inputs["opacities"], inputs["view_matrix"])

    key = (L, caps, offs)
    if key not in _compile_cache:
        _compile_cache[key] = _build_program(L, caps, offs, blocks, CB)
    nc = _compile_cache[key]

    from concourse.bass_utils import run_bass_kernel_spmd
    kw = {}
    if trace:
        kw = dict(trace=True,
                  trace_cores=trace_cores or list(range(NCORES)))
    res = run_bass_kernel_spmd(nc, in_maps, core_ids=list(range(NCORES)), **kw)
    return _assemble(res.results, core_tiles, caps), res


def kernel(**inputs) -> np.ndarray:
    out, _ = _run(inputs, trace=False)
    return out


# revision 13
# speedup vs baseline: 1.7602x; 1.0541x over previous
"""Differentiable Gaussian renderer as a Trainium2 Bass kernel.

Strategy (self-contained; shapes hardcoded from the problem spec):
  - 8 NeuronCores. The 256x256 image is cut into 512 tiles of 8x16 = 128
    pixels (the SBUF partition count). Tiles are assigned to cores by a
    count-balancing serpentine over per-tile culled-gaussian counts, so
    the SPMD rank-max slot capacities are near the mean (not the max of
    a spatial band).
  - Host prep (numpy, float64): project gaussians, depth-sort, precise
    point-to-rectangle mahalanobis culling per (tile), pack each core's
    64 slots as [sep][g0..gC-1] segments along the free dimension
    (identical capacities on all cores -> one NEFF runs SPMD), never
    crossing a 512-col PSUM bank.
  - Device, per 512-col bank:
      Q      = gm.T @ F          (one fp32r matmul, PSUM)
      alpha  = Exp(Q)            (ACT, in-place in PSUM)
      omr    = 1 - alpha         (ACT Identity scale=-1 bias=+1 -> f16 SBUF)
      om     = max(omr, 0.01)    (DVE in-place f16, 4x mode)
      T      = scan(om, inj)     (DVE tensor_tensor_scan mult/max; inj
                                  resets T to 1 at separators; chained
                                  across banks via init)
    Colors use Abel summation: sum_c w[c]*col[c] == sum_c T[c]*dcol[c]
    with dcol[c] = col[c+1]-col[c], so T is transposed directly
    (per 128-col block on the PE) and one small matmul per block
    accumulates into the output PSUM; no explicit w tensor.
  - Output is clipped and DMA'd in two chunks so the first chunk
    overlaps the tail of compute. Host unscrambles [128, 192] per-core
    outputs into [3, 256, 256].
"""

import math
import numpy as np

H = W = 256
FX = FY = 300.0
CX = CY = 128.0
NEAR, FAR = 0.01, 100.0
TR, TC = 8, 16          # pixel tile shape (rows x cols); TR*TC == 128
NTY, NTX = H // TR, W // TC     # 32 x 16 = 512 tiles
NSLOTS = 64             # slots per core (512 tiles / 8 cores)
NCORES = 8
QCUT = 5.5              # keep (gaussian, tile) if max_tile Q + log(opacity) > -QCUT
F_PAD = -88.0           # Q constant for separator / padding columns -> exp ~ 0
BANK = 512

_compile_cache: dict = {}


def _host_prep(positions, scales, rotations, colors, opacities, view_matrix):
    N = positions.shape[0]
    f32 = np.float32
    f16 = np.float16

    # ---- depth sort exactly as the fp32 reference does ----
    pts_h32 = np.concatenate(
        [positions.astype(f32), np.ones((N, 1), f32)], axis=1)
    pcam32 = pts_h32 @ view_matrix.astype(f32).T
    x32, y32, z32 = pcam32[:, 0], pcam32[:, 1], pcam32[:, 2]
    depths32 = -z32
    order = np.argsort(depths32, kind="stable")

    # visibility mask in fp32 (must match reference's boundary decisions)
    z_safe32 = (np.clip(np.abs(z32), 0.01, None) *
                np.sign(z32 + f32(1e-8))).astype(f32)
    u32 = (f32(FX) * x32 / -z_safe32 + f32(CX)).astype(f32)
    v32 = (f32(FY) * -y32 / -z_safe32 + f32(CY)).astype(f32)
    vis = ((depths32 > NEAR) & (depths32 < FAR)
           & (u32 > -100) & (u32 < W + 100)
           & (v32 > -100) & (v32 < H + 100))

    # ---- float64 versions of the per-gaussian quantities ----
    pos = positions.astype(np.float64)
    sc = scales.astype(np.float64)
    rot = rotations.astype(np.float64)
    vm = view_matrix.astype(np.float64)
    q = rot / np.linalg.norm(rot, axis=-1, keepdims=True)
    qw, qx, qy, qz = q[:, 0], q[:, 1], q[:, 2], q[:, 3]
    Rm = np.stack([
        1 - 2*qy*qy - 2*qz*qz, 2*qx*qy - 2*qw*qz, 2*qx*qz + 2*qw*qy,
        2*qx*qy + 2*qw*qz, 1 - 2*qx*qx - 2*qz*qz, 2*qy*qz - 2*qw*qx,
        2*qx*qz - 2*qw*qy, 2*qy*qz + 2*qw*qx, 1 - 2*qx*qx - 2*qy*qy,
    ], axis=-1).reshape(N, 3, 3)
    pts = np.concatenate([pos, np.ones((N, 1))], 1) @ vm.T
    X, Y, Z = pts[:, 0], pts[:, 1], pts[:, 2]
    Rcam = np.einsum('ij,njk->nik', vm[:3, :3], Rm)
    RS = Rcam * sc[:, None, :]
    cov3d = RS @ np.swapaxes(RS, -1, -2)
    z_safe = np.clip(np.abs(Z), 0.01, None) * np.sign(Z + 1e-8)
    z2 = z_safe * z_safe
    J = np.zeros((N, 2, 3))
    J[:, 0, 0] = FX / -z_safe
    J[:, 0, 2] = FX * X / z2
    J[:, 1, 1] = FY / z_safe
    J[:, 1, 2] = FY * Y / z2
    cov2d = np.einsum('nij,njk,nlk->nil', J, cov3d, J)
    u = FX * X / -z_safe + CX
    v = FY * -Y / -z_safe + CY

    # sort everything front-to-back
    u, v, vis = u[order], v[order], vis[order]
    cov2d = cov2d[order]
    opa = opacities.astype(np.float64)[order]
    cols = colors.astype(np.float64)[order]

    a = cov2d[:, 0, 0] + 1e-4
    b = cov2d[:, 0, 1]
    c = cov2d[:, 1, 1] + 1e-4
    det = a * c - b * b
    ia2 = -0.5 * c / det
    ib2 = b / det
    ic2 = -0.5 * a / det
    keepable = vis & (opa > 0)
    logo = np.where(keepable, np.log(np.maximum(opa, 1e-300)), -1e9)

    # ---- precise per-tile culling ----
    # max over the tile rectangle of the concave quadratic Q(p); exact via
    # edge maximization + interior check.
    def qmax_tile(y0, x0):
        inside = (u >= x0) & (u <= x0 + TC - 1) & (v >= y0) & (v <= y0 + TR - 1)
        best = np.full(N, -np.inf)
        for xe in (x0, x0 + TC - 1):
            dx = xe - u
            dy_cl = np.clip(-ib2 * dx / (2 * ic2), y0 - v, y0 + TR - 1 - v)
            best = np.maximum(best, ia2*dx*dx + ib2*dx*dy_cl + ic2*dy_cl*dy_cl)
        for ye in (y0, y0 + TR - 1):
            dy = ye - v
            dx_cl = np.clip(-ib2 * dy / (2 * ia2), x0 - u, x0 + TC - 1 - u)
            best = np.maximum(best, ia2*dx_cl*dx_cl + ib2*dx_cl*dy + ic2*dy*dy)
        return np.where(inside, 0.0, best)

    NT = NTY * NTX                       # 512 global tiles
    keep = np.zeros((NT, N), bool)
    for t in range(NT):
        ty, tx = divmod(t, NTX)
        keep[t] = keepable & (qmax_tile(ty * TR, tx * TC) + logo > -QCUT)
    counts = keep.sum(axis=1)            # [512]

    # ---- balanced tile -> core assignment (serpentine over sorted counts) ----
    tile_order = np.argsort(-counts, kind="stable")
    core_tiles = np.zeros((NCORES, NSLOTS), np.int64)   # rank r -> global tile
    for r in range(NSLOTS):
        blk = tile_order[r * NCORES:(r + 1) * NCORES]
        cores = range(NCORES) if r % 2 == 0 else range(NCORES - 1, -1, -1)
        for t, core in zip(blk, cores):
            core_tiles[core, r] = t
    # rank-wise capacities (max over cores of r-th largest count)
    caps = np.zeros(NSLOTS, np.int64)
    for r in range(NSLOTS):
        caps[r] = max(int(counts[core_tiles[core, r]]) for core in range(NCORES))

    # pack slots as [sep][g...] segments, never crossing a 512-col PSUM bank
    offs = np.full(NSLOTS, -1, np.int64)
    col0 = 0
    for r in range(NSLOTS):
        if caps[r] == 0:
            continue
        seg = int(caps[r]) + 1
        if (col0 % BANK) + seg > BANK:
            col0 = (col0 // BANK + 1) * BANK
        offs[r] = col0
        col0 += seg
    L = int(col0)
    L += L % 2          # fp32r matmul needs an even moving free dim

    # color-matmul blocks: for each 128-col block of L, the (rank-consecutive)
    # slots whose [sep..sep+cap] interval intersects it, plus a block-sparse
    # color-difference matrix [128, 3k] (Abel summation: dcol = col[c+1]-col[c])
    nblocks = -(-L // 128)
    blocks = []          # (bb, m, j0, j1, cb_off)
    cb_off = 0
    for bb in range(nblocks):
        lo, hi = bb * 128, min(bb * 128 + 128, L)
        m = hi - lo
        js = [j for j in range(NSLOTS) if caps[j] > 0
              and offs[j] < hi and offs[j] + caps[j] >= lo]
        if not js:
            continue
        j0, j1 = min(js), max(js)
        assert js == list(range(j0, j1 + 1))
        blocks.append((bb, m, j0, j1, cb_off))
        cb_off += 3 * (j1 - j0 + 1)
    CB = max(cb_off, 1)

    # ---- packed per-core arrays ----
    fmat = np.zeros((NCORES, 6, L), f32)
    fmat[:, 5, :] = F_PAD
    colfull = np.zeros((NCORES, L + 1, 3), f32)   # per-position colors (+1 pad)

    for core in range(NCORES):
        for r in range(NSLOTS):
            if caps[r] == 0:
                continue
            t = int(core_tiles[core, r])
            n = int(counts[t])
            if n == 0:
                continue
            ty, tx = divmod(t, NTX)
            y0, x0 = ty * TR, tx * TC
            x0c = x0 + (TC - 1) / 2.0
            y0c = y0 + (TR - 1) / 2.0
            g = np.where(keep[t])[0]        # sorted (front-to-back)
            up = u[g] - x0c
            vp = v[g] - y0c
            s = int(offs[r]) + 1
            fmat[core, 0, s:s+n] = ia2[g]
            fmat[core, 1, s:s+n] = ib2[g]
            fmat[core, 2, s:s+n] = ic2[g]
            fmat[core, 3, s:s+n] = -2*ia2[g]*up - ib2[g]*vp
            fmat[core, 4, s:s+n] = -2*ic2[g]*vp - ib2[g]*up
            fmat[core, 5, s:s+n] = (ia2[g]*up*up + ib2[g]*up*vp
                                    + ic2[g]*vp*vp + logo[g])
            colfull[core, s:s+n, :] = cols[g, :]

    # Abel: dcol[c] = col[c+1] - col[c]
    dcol = colfull[:, 1:, :] - colfull[:, :-1, :]       # [NCORES, L, 3]
    colblk = np.zeros((NCORES, 128, CB), f16)
    for bb, m, j0, j1, cbo in blocks:
        lo = bb * 128
        for j in range(j0, j1 + 1):
            a0 = max(int(offs[j]), lo)
            a1 = min(int(offs[j]) + int(caps[j]) + 1, lo + m)
            if a0 >= a1:
                continue
            rows = np.arange(a0 - lo, a1 - lo)
            colblk[:, rows, cbo + 3*(j - j0):cbo + 3*(j - j0) + 3] = \
                dcol[:, a0:a1, :].astype(f16)

    # pixel polynomial matrix, shared by every tile and core
    dr, dc = np.divmod(np.arange(128), TC)
    gx = (dc - (TC - 1) / 2.0).astype(f32)
    gy = (dr - (TR - 1) / 2.0).astype(f32)
    gm = np.stack([gx*gx, gx*gy, gy*gy, gx, gy, np.ones(128, f32)]).astype(f32)

    inj = np.zeros(L, f16)
    inj[offs[offs >= 0]] = 1.0
    # scans run per bank with init=0; every bank holding a slot must open
    # with a separator (packing never splits a segment across banks)
    for b in range(0, L, BANK):
        if ((offs >= b) & (offs < b + BANK)).any():
            assert inj[b] == 1.0, f"bank at {b} does not start with a separator"
    inj_rep = np.broadcast_to(inj, (128, L)).copy()

    ident = np.eye(128, dtype=f16)
    in_maps = []
    for core in range(NCORES):
        fmat_all = np.concatenate([gm, fmat[core]], axis=1)   # [6, 128+L] f32
        in_maps.append({
            "fmat": np.ascontiguousarray(fmat_all),
            "colblk": np.ascontiguousarray(colblk[core]),
            "ident": ident,
            "inj": inj_rep,
        })
    caps_t = tuple(int(x) for x in caps)
    offs_t = tuple(int(x) for x in offs)
    return in_maps, L, caps_t, offs_t, core_tiles, blocks, CB


def _choose_split(L, blocks):
    """Pick a clean (non-straddling) block boundary near 65% of L for the
    two-chunk output; returns (split_block_index, j_split) or None."""
    best = None
    for k in range(len(blocks) - 1):
        bb, m, j0, j1, cbo = blocks[k]
        nj0 = blocks[k + 1][2]
        if nj0 > j1:                      # clean boundary
            pos = (bb * 128 + m) / L
            score = abs(pos - 0.65)
            if best is None or score < best[0]:
                best = (score, k, j1 + 1)
    if best is None:
        return None
    return best[1], best[2]


def _build_program(L, caps, offs, blocks, CB):
    import concourse.bacc as bacc
    import concourse.mybir as mybir
    from concourse.tile import TileContext
    from concourse.mybir import AluOpType

    f32 = mybir.dt.float32
    f32r = mybir.dt.float32r
    f16 = mybir.dt.float16
    nc = bacc.Bacc("TRN2", target_bir_lowering=False)
    f_d = nc.dram_tensor("fmat", [6, 128 + L], f32r, kind="ExternalInput")
    cb_d = nc.dram_tensor("colblk", [128, CB], f16, kind="ExternalInput")
    id_d = nc.dram_tensor("ident", [128, 128], f16, kind="ExternalInput")
    inj_d = nc.dram_tensor("inj", [128, L], f16, kind="ExternalInput")
    out_d = nc.dram_tensor("out", [128, 3 * NSLOTS], f32, kind="ExternalOutput")

    banks = []
    c0 = 0
    while c0 < L:
        banks.append((c0, min(c0 + BANK, L)))
        c0 += BANK
    blocks_by_bank: dict[int, list] = {}
    for blk in blocks:
        blocks_by_bank.setdefault(blk[0] // 4, []).append(blk)

    split = _choose_split(L, blocks)
    if split is not None:
        split_k, j_split = split
    else:
        split_k, j_split = len(blocks) - 1, NSLOTS
    # chunk A: slots [0, j_split) finalized after block index split_k
    # chunk B: slots [j_split, NSLOTS)

    with TileContext(nc) as tc:
        with (
            tc.tile_pool(name="const", bufs=1) as cpool,
            tc.tile_pool(name="wts", bufs=3) as wpool,
            tc.tile_pool(name="psum", bufs=3, space="PSUM") as ppool,
            tc.tile_pool(name="trps", bufs=2, space="PSUM") as tpool,
            tc.tile_pool(name="colps", bufs=1, space="PSUM") as opool,
        ):
            # DMA priority order (SP issues serially; ~0.6-1us DGE each):
            # fmat gates the whole pipeline, then bank-0 inj, then the
            # transpose/color constants, then the remaining inj banks.
            fm_all = cpool.tile([6, 128 + L], f32r)
            nc.sync.dma_start(fm_all[:, :], f_d[:, :])
            gm = fm_all[:, 0:128]
            fmat = fm_all[:, 128:128 + L]
            inj = cpool.tile([128, L], f16)
            nc.sync.dma_start(inj[:, banks[0][0]:banks[0][1]],
                              inj_d[:, banks[0][0]:banks[0][1]])
            ident = cpool.tile([128, 128], f16)
            nc.sync.dma_start(ident[:, :], id_d[:, :])
            cb = cpool.tile([128, CB], f16)
            nc.sync.dma_start(cb[:, :], cb_d[:, :])
            for c0, c1 in banks[1:]:
                nc.sync.dma_start(inj[:, c0:c1], inj_d[:, c0:c1])

            om = cpool.tile([128, L], f16)
            Tt = cpool.tile([128, L], f16)
            colb = cpool.tile([128, 3 * NSLOTS], f32)
            nA = 3 * j_split
            nB = 3 * NSLOTS - nA
            colpsA = opool.tile([128, max(nA, 1)], f32, name="colpsA")
            colpsB = opool.tile([128, max(nB, 1)], f32, name="colpsB")
            nc.vector.memset(colpsA[:, :], 0.0)
            nc.vector.memset(colpsB[:, :], 0.0)

            def col_mm(bidx, blk, wT, toff):
                bb, m, j0, j1, cbo = blk
                k3 = 3 * (j1 - j0 + 1)
                # split the rhs columns between the two output chunks
                lo_j, hi_j = j0, j1 + 1
                if lo_j < j_split:
                    hj = min(hi_j, j_split)
                    k3a = 3 * (hj - lo_j)
                    nc.tensor.matmul(colpsA[:, 3 * lo_j:3 * lo_j + k3a],
                                     wT[:m, toff:toff + 128],
                                     cb[:m, cbo:cbo + k3a],
                                     start=False, stop=False,
                                     skip_group_check=True)
                if hi_j > j_split:
                    lj = max(lo_j, j_split)
                    cbo2 = cbo + 3 * (lj - j0)
                    k3b = 3 * (hi_j - lj)
                    nc.tensor.matmul(
                        colpsB[:, 3 * (lj - j_split):3 * (lj - j_split) + k3b],
                        wT[:m, toff:toff + 128],
                        cb[:m, cbo2:cbo2 + k3b],
                        start=False, stop=False,
                        skip_group_check=True)

            LN99 = float(math.log(0.99))
            drain_on_vector = True
            blk_idx = 0
            for bi, (c0, c1) in enumerate(banks):
                n = c1 - c0
                ps = ppool.tile([128, BANK], f32, tag="ps", name="ps")
                nc.tensor.matmul(ps[:, :n],
                                 gm[:, :],
                                 fmat[:, c0:c1],
                                 start=True, stop=True)
                # alpha = exp(Q), in place in PSUM
                nc.scalar.activation(ps[:, :n], ps[:, :n],
                                     mybir.ActivationFunctionType.Exp)
                # omr = 1 - alpha  (ACT affine: Identity(-1*x + 1)) -> f16 SBUF
                nc.scalar.activation(om[:, c0:c1], ps[:, :n],
                                     mybir.ActivationFunctionType.Identity,
                                     bias=1.0, scale=-1.0)
                # om = max(omr, 0.01), in place (f16 4x mode)
                nc.vector.tensor_scalar(om[:, c0:c1], om[:, c0:c1], 0.01,
                                        None, AluOpType.max)
                # Every bank starts with a separator (segments never cross a
                # bank), so bank scans are independent (init=0, no chaining).
                # The scan opcode is DVE-only (Pool codegen rejects it).
                nc.vector.tensor_tensor_scan(Tt[:, c0:c1], om[:, c0:c1],
                                             inj[:, c0:c1], 0.0,
                                             AluOpType.mult, AluOpType.max)
                # color path: transpose T per 128-col block (pairs share one
                # PSUM tile + one drain), then small matmuls accumulate
                blks = blocks_by_bank.get(bi, [])
                for p0 in range(0, len(blks), 2):
                    pair = blks[p0:p0 + 2]
                    trp = tpool.tile([128, 256], f16, tag="trp", name="trp")
                    wT = wpool.tile([128, 256], f16, tag="wT", name="wT")
                    span = 0
                    for t, (bb, m, j0, j1, cbo) in enumerate(pair):
                        lo = bb * 128
                        nc.tensor.transpose(trp[:m, 128 * t:128 * t + 128],
                                            Tt[:, lo:lo + m], ident[:, :])
                        span = 128 * t + 128
                    if drain_on_vector:
                        nc.vector.tensor_copy(wT[:, :span], trp[:, :span])
                    else:
                        nc.scalar.copy(wT[:, :span], trp[:, :span])
                    drain_on_vector = not drain_on_vector
                    for t, blk in enumerate(pair):
                        col_mm(blk_idx, blk, wT, 128 * t)
                        blk_idx += 1
                        if blk_idx == split_k + 1 and nA > 0:
                            # chunk A final: clip + DMA now
                            nc.vector.tensor_scalar(
                                colb[:, :nA], colpsA[:, :nA], 0.0, 1.0,
                                AluOpType.max, AluOpType.min)
                            nc.sync.dma_start(out_d[:, :nA], colb[:, :nA])

            if nB > 0:
                nc.vector.tensor_scalar(colb[:, nA:], colpsB[:, :nB], 0.0, 1.0,
                                        AluOpType.max, AluOpType.min)
                nc.scalar.dma_start(out_d[:, nA:], colb[:, nA:])
            if split_k + 1 > len(blocks) and nA > 0:
                raise AssertionError("chunk A never finalized")
    nc.finalize()
    return nc


def _assemble(results, core_tiles, caps):
    out = np.zeros((3, H, W), np.float32)
    dr, dc = np.divmod(np.arange(128), TC)
    for core in range(NCORES):
        o = results[core]["out"]          # [128, 192]
        for r in range(NSLOTS):
            if caps[r] == 0:
                continue
            t = int(core_tiles[core, r])
            ty, tx = divmod(t, NTX)
            y0, x0 = ty * TR, tx * TC
            for ch in range(3):
                out[ch, y0 + dr, x0 + dc] = o[:, 3 * r + ch]
    return out


def _run(inputs, trace=False, trace_cores=None):
    in_maps, L, caps, offs, core_tiles, blocks, CB = _host_prep(
        inputs["positions"], inputs["scales"], inputs["rotations"],
        inputs["colors"], inputs["opacities"], inputs["view_matrix"])

    key = (L, caps, offs)
    if key not in _compile_cache:
        _compile_cache[key] = _build_program(L, caps, offs, blocks, CB)
    nc = _compile_cache[key]

    from concourse.bass_utils import run_bass_kernel_spmd
    kw = {}
    if trace:
        kw = dict(trace=True,
                  trace_cores=trace_cores or list(range(NCORES)))
    res = run_bass_kernel_spmd(nc, in_maps, core_ids=list(range(NCORES)), **kw)
    return _assemble(res.results, core_tiles, caps), res


def kernel(**inputs) -> np.ndarray:
    out, _ = _run(inputs, trace=False)
    return out
